# revision 1
# baseline (speedup 1.0000x reference)
"""AI4DEM contact-force stencil kernel for 8 Trainium2 NeuronCores — v2.

v2 over the staged baseline: engine rebalance of the 62-offset pair chain.
- DVE kept for: position diffs, q products, vd partial sums, recip custom,
  new fused WGATE custom ((vd*inv2)*(inv2>1/CS^2)) and the t=G*d muls.
- ACT absorbs: the three Squares, Sqrt, a Relu-gated spring
  Sg = Relu(KNCS/r - KN) (exact: active <=> r<CS), and the neighbor-variant
  constant adds (Copy with float bias).
- Pool (gpsimd) absorbs: r2 partial add + the three velocity-diff subs.
- PE unchanged: direct+scatter force accumulation via (shifted-)identity
  matmuls into PSUM.
Gating restructure (exact algebra, no Sign/affine/+KN ops):
  G = Relu(invK - KN) - (vd*inv2)*(inv2 > 1/CS2)
"""
import math
import numpy as np
import ml_dtypes

import concourse.bass as bass
import concourse.mybir as mybir
from concourse import bacc, dve_ops
from concourse.dve_spec import Spec, Src0, Src1, C0, lower, _has_src1, relu, sq
from concourse.dve_spec import C1 as DC1
from concourse.dve_uop import DveOpSpec
from concourse.tile import TileContext
from concourse.bass_utils import run_bass_kernel_spmd
from concourse.dve_ops import RECIP_APPROX_FAST_CONSTS, RECIPROCAL_APPROX_FAST

Alu = mybir.AluOpType
Act = mybir.ActivationFunctionType
F32 = mybir.dt.float32
BF16 = mybir.dt.bfloat16
FP16 = mybir.dt.float16

# --- physics constants (match reference) ---
CS = 0.05
KN = 600000.0
DT = 0.001
GRAV = 9.8
_ALPHA = -math.log(0.5) / math.pi
_GAMMA = _ALPHA / math.sqrt(_ALPHA**2 + 1.0)
MASS = 4.0 / 3.0 * 3.1416 * CS**3 * 2700.0
ETA = 2.0 * _GAMMA * math.sqrt(KN * MASS / 2.0)
KNCS = KN * CS
SCL = KNCS * KNCS          # ACT Sqrt scale: sqrt(SCL*inv2) = KNCS/r
CS2 = float(np.float32(CS) * np.float32(CS))
INV_CS2 = float(1.0 / CS2)
C1 = DT / MASS

NZ_G, Y, X = 128, 128, 128
XP, YP, ZS, ZOWN = 136, 132, 20, 16
NB, BS = 2, 8
N_CORES = 8

OFFS = [(dz, dy, dx)
        for dz in range(0, 3) for dy in range(-2, 3) for dx in range(-2, 3)
        if (dz > 0) or (dz == 0 and dy > 0) or (dz == 0 and dy == 0 and dx > 0)]
DY_ORDER = [0, 1, 2, -1, -2]
DY_GROUPS = {dy: [o for o in OFFS if o[1] == dy] for dy in DY_ORDER}
N_OFFS = len(OFFS)
POS = ["pxm", "yg", "zg"]
VELE = ["vxe", "vye", "vze"]


def _register_dve_op(name, spec, subdim=False):
    if name in dve_ops._SUB_OPCODE_FOR_NAME:
        return next(o for o in dve_ops.OPS if o.name == name)
    row = dve_ops._CUSTOM_DVE_ROW_BASE + len(dve_ops.OPS)
    assert row < 0x20
    shas = {}
    for ver in ("v3", "v4"):
        try:
            uops = lower(spec, ver=ver)
            shas[ver] = DveOpSpec(name=name, opcode=row, uops=uops,
                                  rd1_en=_has_src1(spec)).sha(ver)
        except ValueError:
            pass
    op = dve_ops.DveOp(name, spec, subdim=subdim, uops_sha=shas)
    dve_ops.OPS.append(op)
    dve_ops.CUSTOM_DVE_SPECS[name] = spec
    dve_ops._SUB_OPCODE_FOR_NAME[name] = row
    return op


# wg = (vd * inv2) * (inv2 > 1/CS2): gated damping term in one DVE inst
WGATE = _register_dve_op(
    "WGATE_AI4",
    Spec(
        body=(Src0 * Src1) * (Src1 > C0),
        reference=lambda in0, in1, s0, s1, imm2: (
            (in0.astype(np.float32) * in1) * (in1 > s0)
        ).astype(np.float32),
    ),
)


# G = relu(invK - KN) - ((vd * invK^2) * (1/SCL)) * (invK > KN)
# (one inst for spring + gated damping + combine; gate invK>KN == r<CS)
GFUSE = _register_dve_op(
    "GFUSE_AI4",
    Spec(
        body=relu(Src1 - C0) - ((Src0 * sq(Src1)) * DC1) * (Src1 > C0),
        reference=lambda in0, in1, s0, s1, imm2: (
            np.maximum(in1.astype(np.float32) - s0, 0)
            - ((in0.astype(np.float32) * in1.astype(np.float32) ** 2) * s1)
            * (in1 > s0)
        ).astype(np.float32),
    ),
)


_HOST_CONST = {}


def _rmod_cache():
    if "rmod" not in _HOST_CONST:
        zi, yi, xi = np.meshgrid(np.arange(NZ_G + 4), np.arange(YP),
                                 np.arange(XP), indexing="ij")
        _HOST_CONST["rmod"] = ((zi % 5) * 25 + (yi % 5) * 5 + (xi % 5)
                               + 1).astype(np.float32)
    return _HOST_CONST["rmod"]


def _idents_cache():
    if "idents" not in _HOST_CONST:
        _HOST_CONST["idents"] = np.stack(
            [np.eye(128, dtype=np.float32)] +
            [-np.eye(128, k=dy, dtype=np.float32) for dy in range(-2, 3)]
        ).astype(ml_dtypes.bfloat16)
    return _HOST_CONST["idents"]


def host_prepare(inputs):
    """Full inputs -> list of 8 per-core in_maps."""
    full = {k: np.asarray(v, dtype=np.float32).reshape(NZ_G, Y, X)
            for k, v in inputs.items()}

    def pad(a):
        return np.pad(a, ((2, 2), (2, 2), (4, 4)))

    xg = pad(full["x_grid"]); yg = pad(full["y_grid"]); zg = pad(full["z_grid"])
    vx = pad(full["vx_grid"]); vy = pad(full["vy_grid"]); vz = pad(full["vz_grid"])
    m = pad(full["mask_grid"])
    rmod = _rmod_cache()
    half = np.float32(0.5)
    disp = np.float32(5.0) * (np.float32(1.0) - m) * rmod
    pxm16 = ((xg - half) + disp).astype(np.float16)
    yg16 = (yg - half).astype(np.float16)
    zg16 = (zg - half).astype(np.float16)
    pxm = xg
    eta = np.float32(ETA)
    vxe = (eta * vx).astype(ml_dtypes.bfloat16)
    vye = (eta * vy).astype(ml_dtypes.bfloat16)
    vze = (eta * vz).astype(ml_dtypes.bfloat16)

    idents = _idents_cache()

    arrs = {"pxm": pxm, "yg": yg, "zg": zg, "vx": vx, "vy": vy, "vz": vz,
            "mg": m, "mg16": m.astype(ml_dtypes.bfloat16),
            "vxe": vxe, "vye": vye, "vze": vze,
            "pxm16": pxm16, "yg16": yg16, "zg16": zg16}
    in_maps = []
    for c in range(N_CORES):
        sl = slice(c * ZOWN, c * ZOWN + ZS)
        im = {k: np.ascontiguousarray(a[sl].transpose(1, 0, 2))  # [YP, ZS, XP]
              for k, a in arrs.items()}
        im["idents"] = idents
        in_maps.append(im)
    return in_maps


def build_nc():
    nc = bacc.Bacc("TRN2", target_bir_lowering=False, debug=False,
                   num_devices=N_CORES)
    # const APs for ACT biases (init-time only)
    for val in (-KN,):
        t = nc.alloc_sbuf_tensor(f"const-float32-{val}", [128, 1], F32)
        nc.gpsimd.memset(t.ap(), val)
        nc.const_aps.aps[(F32, val)] = t.ap()
    nc.all_engine_barrier()
    d = {}
    for k in POS + ["vx", "vy", "vz"]:
        d[k] = nc.dram_tensor(k, [YP, ZS, XP], F32, kind="ExternalInput")
    for k in VELE + ["mg16"]:
        d[k] = nc.dram_tensor(k, [YP, ZS, XP], BF16, kind="ExternalInput")
    for k in ["pxm16", "yg16", "zg16"]:
        d[k] = nc.dram_tensor(k, [YP, ZS, XP], FP16, kind="ExternalInput")
    idents_d = nc.dram_tensor("idents", [6, 128, 128], BF16, kind="ExternalInput")
    out_d = nc.dram_tensor("out", [6, Y, ZOWN, X], F32, kind="ExternalOutput")

    XL, XH = 2, 134            # fixed chain x-window (all ops, width 132)
    _rc = RECIP_APPROX_FAST_CONSTS

    with TileContext(nc) as tc:
        with tc.tile_pool(name="const", bufs=1) as cpool, \
             tc.tile_pool(name="cent", bufs=1) as gpool, \
             tc.tile_pool(name="shift", bufs=2) as spool, \
             tc.tile_pool(name="pvar", bufs=1) as pvpool, \
             tc.tile_pool(name="vel", bufs=2) as vpool, \
             tc.tile_pool(name="tmp", bufs=2) as tpool, \
             tc.tile_pool(name="tmp2", bufs=2) as tpool2, \
             tc.tile_pool(name="poolout", bufs=2) as plpool, \
             tc.tile_pool(name="tmpB", bufs=2) as tbpool, \
             tc.tile_pool(name="stg", bufs=1) as stpool, \
             tc.tile_pool(name="psum", bufs=1, space="PSUM") as ppool:

            idt = cpool.tile([128, 6, 128], BF16, tag="idents")
            for j in range(6):
                nc.sync.dma_start(out=idt[:, j, :], in_=idents_d[j])

            for b in range(NB):
                # center tiles: rows [2,130), z [8b, 8b+12)
                C = {}
                for k in ["pxm16", "yg16", "zg16"]:
                    t = gpool.tile([128, 12, XP], FP16, tag=f"c_{k}")
                    nc.sync.dma_start(out=t[:], in_=d[k][2:130, 8 * b:8 * b + 12, :])
                    C[k] = t
                t = gpool.tile([128, 8, XP], BF16, tag="c_mg")
                nc.sync.dma_start(out=t[:],
                                  in_=d["mg16"][2:130, 8 * b + 2:8 * b + 10, :])
                C["mg"] = t
                for k in VELE:
                    t = gpool.tile([128, 12, XP], BF16, tag=f"c_{k}")
                    nc.sync.dma_start(out=t[:], in_=d[k][2:130, 8 * b:8 * b + 12, :])
                    C[k] = t

                FX = ppool.tile([128, BS, X], F32, tag="fx")
                FY = ppool.tile([128, BS, X], F32, tag="fy")
                FZ = ppool.tile([128, BS, X], F32, tag="fz")
                started = set()
                n_done = 0

                for dy in DY_ORDER:
                    # fp16 position neighbors: staged for dy!=0, else views of
                    # the fp16 center tiles; per-group variant ops bake the d*CS
                    # constants into neighbor variants (odd dx pre-shifted 1
                    # elem for 4B alignment). Variant adds run on ACT (Copy
                    # with float bias) to keep DVE free.
                    if dy == 0:
                        S16 = {k: C[k][:, 2:12, :]
                               for k in ["pxm16", "yg16", "zg16"]}
                    else:
                        S16 = {}
                        for k in ["pxm16", "yg16", "zg16"]:
                            t = spool.tile([128, 10, XP], FP16, tag=f"s_{k}")
                            nc.sync.dma_start(
                                out=t[:],
                                in_=d[k][2 + dy:130 + dy, 2 + 8 * b:12 + 8 * b, :])
                            S16[k] = t[:]
                    PV = {}
                    for dx in range(-2, 3):
                        if dx == 0:
                            PV[0] = S16["pxm16"]
                            continue
                        pv = pvpool.tile([128, 10, XP], FP16, tag=f"pv{dx}")
                        nc.scalar.activation(pv[:], S16["pxm16"], Act.Copy,
                                             bias=float(dx * CS))
                        PV[dx] = pv
                    # even-parity y/z variants (read by even-dx offsets)
                    if dy == 0:
                        yv = S16["yg16"]
                    else:
                        yvt = pvpool.tile([128, 10, XP], FP16, tag="yv")
                        nc.scalar.activation(yvt[:], S16["yg16"], Act.Copy,
                                             bias=float(dy * CS))
                        yv = yvt[:]
                    ZV = {}
                    for dz in sorted({o[0] for o in DY_GROUPS[dy]}):
                        if dz == 0:
                            ZV[0] = S16["zg16"]
                        else:
                            zv = pvpool.tile([128, 10, XP], FP16, tag=f"zv{dz}")
                            nc.scalar.activation(zv[:], S16["zg16"], Act.Copy,
                                                 bias=float(dz * CS))
                            ZV[dz] = zv[:]
                    # velocity neighbors (bf16): staged for every dy
                    # (2B-aligned odd-dx windows read directly; HW-validated)
                    SVE = {}
                    for k in VELE:
                        te = vpool.tile([128, 10, XP], BF16, tag=f"se_{k}")
                        nc.sync.dma_start(
                            out=te[:],
                            in_=d[k][2 + dy:130 + dy, 2 + 8 * b:12 + 8 * b, :])
                        SVE[k] = te

                    for (dz, _dy, dx) in DY_GROUPS[dy]:
                        nz = BS + dz
                        zc = 2 - dz
                        # minimal x-window: union of direct [4:132) and
                        # scatter [4-dx:132-dx) consumer reads
                        XLo = 2 if dx > 0 else 4
                        XHo = XLo + (128 if dx == 0 else 130)

                        def cw(t):      # center window
                            return t[:, zc:zc + nz, XLo:XHo]

                        def nwv(k):     # velocity-neighbor window
                            return SVE[k][:, 0:nz, XLo + dx:XHo + dx]

                        def tmp(tag, dt=BF16):
                            t = tpool.tile([128, 10, XP], dt, tag=tag)
                            return t

                        def tmp2(tag, dt=BF16):
                            t = tpool2.tile([128, 10, XP], dt, tag=tag)
                            return t

                        def tmpP(tag, dt=BF16):
                            t = plpool.tile([128, 10, XP], dt, tag=tag)
                            return t

                        def tmpB(tag, dt=BF16):
                            t = tbpool.tile([128, 10, XP], dt, tag=tag)
                            return t

                        def win(t):
                            return t[:, 0:nz, XLo:XHo]

                        nby = yv[:, 0:nz, XLo + dx:XHo + dx]
                        nbz = ZV[dz][:, 0:nz, XLo + dx:XHo + dx]
                        dX = tmp2("dX"); dY_ = tmp2("dY"); dZ = tmp2("dZ")
                        nc.vector.tensor_sub(
                            win(dX), C["pxm16"][:, zc:zc + nz, XLo:XHo],
                            PV[dx][:, 0:nz, XLo + dx:XHo + dx])
                        nc.vector.tensor_sub(
                            win(dY_), C["yg16"][:, zc:zc + nz, XLo:XHo], nby)
                        nc.vector.tensor_sub(
                            win(dZ), C["zg16"][:, zc:zc + nz, XLo:XHo], nbz)

                        # r2 = dX^2+dY^2+dZ^2 (squares on ACT, partial add on
                        # Pool), clamp via fused 4x tensor_scalar
                        x2 = tmpB("x2"); y2 = tmpB("y2"); z2 = tmp("z2")
                        nc.scalar.activation(win(x2), win(dX), Act.Square)
                        nc.scalar.activation(win(y2), win(dY_), Act.Square)
                        nc.scalar.activation(win(z2), win(dZ), Act.Square)
                        r2a = tmp("r2a")
                        if n_done % 2 == 1:
                            nc.vector.tensor_add(win(r2a), win(x2), win(y2))
                        else:
                            nc.gpsimd.tensor_tensor(win(r2a), win(x2), win(y2),
                                                    op=Alu.add)
                        r2 = tmp("r2")
                        nc.vector.tensor_add(win(r2), win(r2a), win(z2))
                        nc.vector.tensor_scalar(win(r2), win(r2), 1e-20, None,
                                                op0=Alu.max)
                        inv2 = tmp2("inv2")
                        nc.vector._custom_dve(RECIPROCAL_APPROX_FAST,
                                              out=win(inv2), in0=win(r2),
                                              s0=_rc["s0"], s1=_rc["s1"],
                                              imm2=_rc["imm2"])
                        # gated spring on ACT: Sg = Relu(KNCS/r - KN)
                        invK = tmpB("invK")
                        nc.scalar.activation(win(invK), win(inv2), Act.Sqrt,
                                             scale=SCL)


                        # velocity diffs on Pool, products + partial sums on DVE
                        dvx = tmpP("dvx"); dvy = tmpP("dvy"); dvz = tmpP("dvz")
                        nc.gpsimd.tensor_tensor(win(dvx), cw(C["vxe"]),
                                                nwv("vxe"), op=Alu.subtract)
                        nc.gpsimd.tensor_tensor(win(dvy), cw(C["vye"]),
                                                nwv("vye"), op=Alu.subtract)
                        nc.gpsimd.tensor_tensor(win(dvz), cw(C["vze"]),
                                                nwv("vze"), op=Alu.subtract)
                        vp1 = tmp2("vp1")
                        nc.vector.tensor_mul(win(vp1), win(dvx), win(dX))
                        dv = tmp("dv")
                        nc.vector.tensor_mul(win(dv), win(dvy), win(dY_))
                        nc.vector.tensor_add(win(vp1), win(vp1), win(dv))
                        dv2 = tmp("dv2")
                        nc.vector.tensor_mul(win(dv2), win(dvz), win(dZ))
                        nc.vector.tensor_add(win(vp1), win(vp1), win(dv2))
                        # G = relu(invK-KN) - (vd*invK^2/SCL)*(invK>KN)
                        G = tmp("G")
                        nc.vector._custom_dve(GFUSE, out=win(G),
                                              in0=win(vp1), in1=win(invK),
                                              s0=KN, s1=float(1.0 / SCL))

                        tX = tmpB("tX"); tY = tmpB("tY"); tZ = tmpB("tZ")
                        nc.vector.tensor_mul(win(tX), win(G), win(dX))
                        nc.vector.tensor_mul(win(tY), win(G), win(dY_))
                        if n_done % 2 == 0:
                            nc.vector.tensor_mul(win(tZ), win(G), win(dZ))
                        else:
                            nc.gpsimd.tensor_tensor(win(tZ), win(G), win(dZ),
                                                    op=Alu.mult)

                        last = n_done == N_OFFS - 1
                        for (F, t) in ((FX, tX), (FY, tY), (FZ, tZ)):
                            for half in (0, 1):
                                zr = 4 * half
                                key = (id(F), half)
                                st = key not in started
                                started.add(key)
                                # direct: F[y,z',x'] += t[y, z'+dz, x'+4]
                                nc.tensor.matmul(
                                    F[:, zr:zr + 4, :],
                                    idt[:, 0, :],
                                    t[:, dz + zr:dz + zr + 4, 4:132],
                                    start=st, stop=False)
                                # scatter: F[y,z',x'] -= t[y-dy, z', x'+4-dx]
                                nc.tensor.matmul(
                                    F[:, zr:zr + 4, :],
                                    idt[:, 1 + (dy + 2), :],
                                    t[:, zr:zr + 4, 4 - dx:132 - dx],
                                    start=False, stop=last)
                        n_done += 1

                # ---- integration (pos/vel f32 loaded on demand) ----
                zv_ = slice(0, 8)
                xs = slice(4, 4 + X)
                mw = C["mg"][:, zv_, xs]
                for ci, (F, vname, pname) in enumerate(
                        ((FX, "vx", "pxm"), (FY, "vy", "yg"), (FZ, "vz", "zg"))):
                    vt = stpool.tile([128, BS, X], F32, tag="vt")
                    nc.sync.dma_start(
                        out=vt[:],
                        in_=d[vname][2:130, 8 * b + 2:8 * b + 10, 4:132])
                    pt = stpool.tile([128, BS, X], F32, tag="pt")
                    nc.sync.dma_start(
                        out=pt[:],
                        in_=d[pname][2:130, 8 * b + 2:8 * b + 10, 4:132])
                    scr = stpool.tile([128, BS, X], F32, tag="scr")
                    nc.vector.scalar_tensor_tensor(
                        scr[:], F[:], C1, vt[:],
                        op0=Alu.mult, op1=Alu.add)
                    stv = stpool.tile([128, BS, X], F32, tag="stg")
                    if ci == 2:
                        nc.vector.scalar_tensor_tensor(
                            stv[:], scr[:], GRAV * DT, mw,
                            op0=Alu.subtract, op1=Alu.mult)
                    else:
                        nc.vector.tensor_mul(stv[:], scr[:], mw)
                    nc.sync.dma_start(out=out_d[3 + ci, :, 8 * b:8 * b + BS, :],
                                      in_=stv[:])
                    scr2 = stpool.tile([128, BS, X], F32, tag="scr")
                    nc.vector.scalar_tensor_tensor(
                        scr2[:], stv[:], DT, pt[:],
                        op0=Alu.mult, op1=Alu.add)
                    stp = stpool.tile([128, BS, X], F32, tag="stg")
                    nc.vector.tensor_mul(stp[:], scr2[:], mw)
                    nc.sync.dma_start(out=out_d[ci, :, 8 * b:8 * b + BS, :],
                                      in_=stp[:])
    nc.compile()
    return nc


_NC_CACHE = []


def get_nc():
    if not _NC_CACHE:
        _NC_CACHE.append(build_nc())
    return _NC_CACHE[0]


def assemble(core_outs):
    full = np.concatenate([o.transpose(0, 2, 1, 3) for o in core_outs], axis=1)
    return full.reshape(6, 1, 1, NZ_G, Y, X)


def kernel(**inputs):
    nc = get_nc()
    in_maps = host_prepare(inputs)
    res = run_bass_kernel_spmd(nc, in_maps, core_ids=list(range(N_CORES)))
    return assemble([res.results[c]["out"] for c in range(N_CORES)])


if __name__ == "__main__":
    rng = np.random.default_rng(0)
    shp = (1, 1, NZ_G, Y, X)
    inputs = {
        "x_grid": rng.random(shp, dtype=np.float32),
        "y_grid": rng.random(shp, dtype=np.float32),
        "z_grid": rng.random(shp, dtype=np.float32),
        "vx_grid": rng.standard_normal(shp, dtype=np.float32),
        "vy_grid": rng.standard_normal(shp, dtype=np.float32),
        "vz_grid": rng.standard_normal(shp, dtype=np.float32),
        "mask_grid": np.ones(shp, dtype=np.float32),
    }
    out = kernel(**inputs)
    print("out shape:", out.shape, "finite:", np.isfinite(out).all())



# revision 11
# speedup vs baseline: 1.3052x; 1.3052x over previous
"""AI4DEM contact-force stencil kernel for 8 Trainium2 NeuronCores — v2.

v2 over the staged baseline: engine rebalance of the 62-offset pair chain.
- DVE kept for: position diffs, q products, vd partial sums, recip custom,
  new fused WGATE custom ((vd*inv2)*(inv2>1/CS^2)) and the t=G*d muls.
- ACT absorbs: the three Squares, Sqrt, a Relu-gated spring
  Sg = Relu(KNCS/r - KN) (exact: active <=> r<CS), and the neighbor-variant
  constant adds (Copy with float bias).
- Pool (gpsimd) absorbs: r2 partial add + the three velocity-diff subs.
- PE unchanged: direct+scatter force accumulation via (shifted-)identity
  matmuls into PSUM.
Gating restructure (exact algebra, no Sign/affine/+KN ops):
  G = Relu(invK - KN) - (vd*inv2)*(inv2 > 1/CS2)
"""
import math
import numpy as np
import ml_dtypes

import concourse.bass as bass
import concourse.mybir as mybir
from concourse import bacc, dve_ops
from concourse.dve_spec import Spec, Src0, Src1, C0, lower, _has_src1, relu, sq
from concourse.dve_spec import C1 as DC1
from concourse.dve_uop import DveOpSpec
from concourse.tile import TileContext
from concourse.bass_utils import run_bass_kernel_spmd
from concourse.dve_ops import RECIP_APPROX_FAST_CONSTS, RECIPROCAL_APPROX_FAST

Alu = mybir.AluOpType
Act = mybir.ActivationFunctionType
F32 = mybir.dt.float32
BF16 = mybir.dt.bfloat16
FP16 = mybir.dt.float16

# --- physics constants (match reference) ---
CS = 0.05
KN = 600000.0
DT = 0.001
GRAV = 9.8
_ALPHA = -math.log(0.5) / math.pi
_GAMMA = _ALPHA / math.sqrt(_ALPHA**2 + 1.0)
MASS = 4.0 / 3.0 * 3.1416 * CS**3 * 2700.0
ETA = 2.0 * _GAMMA * math.sqrt(KN * MASS / 2.0)
KNCS = KN * CS
SCL = KNCS * KNCS          # ACT Sqrt scale: sqrt(SCL*inv2) = KNCS/r
CS2 = float(np.float32(CS) * np.float32(CS))
INV_CS2 = float(1.0 / CS2)
C1 = DT / MASS

NZ_G, Y, X = 128, 128, 128
XP, YP, ZS, ZOWN = 136, 132, 20, 16
NB, BS = 2, 8
N_CORES = 8

OFFS = [(dz, dy, dx)
        for dz in range(0, 3) for dy in range(-2, 3) for dx in range(-2, 3)
        if (dz > 0) or (dz == 0 and dy > 0) or (dz == 0 and dy == 0 and dx > 0)]
DY_ORDER = [0, 1, 2, -1, -2]
DY_GROUPS = {dy: [o for o in OFFS if o[1] == dy] for dy in DY_ORDER}
N_OFFS = len(OFFS)
POS = ["pxm", "yg", "zg"]
VELE = ["vxe", "vye", "vze"]


def _register_dve_op(name, spec, subdim=False):
    if name in dve_ops._SUB_OPCODE_FOR_NAME:
        return next(o for o in dve_ops.OPS if o.name == name)
    row = dve_ops._CUSTOM_DVE_ROW_BASE + len(dve_ops.OPS)
    assert row < 0x20
    shas = {}
    for ver in ("v3", "v4"):
        try:
            uops = lower(spec, ver=ver)
            shas[ver] = DveOpSpec(name=name, opcode=row, uops=uops,
                                  rd1_en=_has_src1(spec)).sha(ver)
        except ValueError:
            pass
    op = dve_ops.DveOp(name, spec, subdim=subdim, uops_sha=shas)
    dve_ops.OPS.append(op)
    dve_ops.CUSTOM_DVE_SPECS[name] = spec
    dve_ops._SUB_OPCODE_FOR_NAME[name] = row
    return op


# wg = (vd * inv2) * (inv2 > 1/CS2): gated damping term in one DVE inst
WGATE = _register_dve_op(
    "WGATE_AI4",
    Spec(
        body=(Src0 * Src1) * (Src1 > C0),
        reference=lambda in0, in1, s0, s1, imm2: (
            (in0.astype(np.float32) * in1) * (in1 > s0)
        ).astype(np.float32),
    ),
)


# G = relu(invK - KN) - ((vd * invK^2) * (1/SCL)) * (invK > KN)
# (one inst for spring + gated damping + combine; gate invK>KN == r<CS)
GFUSE = _register_dve_op(
    "GFUSE_AI4",
    Spec(
        body=relu(Src1 - C0) - ((Src0 * sq(Src1)) * DC1) * (Src1 > C0),
        reference=lambda in0, in1, s0, s1, imm2: (
            np.maximum(in1.astype(np.float32) - s0, 0)
            - ((in0.astype(np.float32) * in1.astype(np.float32) ** 2) * s1)
            * (in1 > s0)
        ).astype(np.float32),
    ),
)


_HOST_CONST = {}


def _rmod_cache():
    if "rmod" not in _HOST_CONST:
        zi, yi, xi = np.meshgrid(np.arange(NZ_G + 4), np.arange(YP),
                                 np.arange(XP), indexing="ij")
        _HOST_CONST["rmod"] = ((zi % 5) * 25 + (yi % 5) * 5 + (xi % 5)
                               + 1).astype(np.float32)
    return _HOST_CONST["rmod"]


def _idents_cache():
    if "idents" not in _HOST_CONST:
        _HOST_CONST["idents"] = np.stack(
            [np.eye(128, dtype=np.float32)] +
            [-np.eye(128, k=dy, dtype=np.float32) for dy in range(-2, 3)]
        ).astype(ml_dtypes.bfloat16)
    return _HOST_CONST["idents"]


def host_prepare(inputs):
    """Full inputs -> list of 8 per-core in_maps."""
    full = {k: np.asarray(v, dtype=np.float32).reshape(NZ_G, Y, X)
            for k, v in inputs.items()}

    def pad(a):
        return np.pad(a, ((2, 2), (2, 2), (4, 4)))

    xg = pad(full["x_grid"]); yg = pad(full["y_grid"]); zg = pad(full["z_grid"])
    vx = pad(full["vx_grid"]); vy = pad(full["vy_grid"]); vz = pad(full["vz_grid"])
    m = pad(full["mask_grid"])
    rmod = _rmod_cache()
    half = np.float32(0.5)
    disp = np.float32(5.0) * (np.float32(1.0) - m) * rmod
    pxm16 = ((xg - half) + disp).astype(np.float16)
    yg16 = (yg - half).astype(np.float16)
    zg16 = (zg - half).astype(np.float16)
    pxm = xg
    eta = np.float32(ETA)
    vxe = (eta * vx).astype(ml_dtypes.bfloat16)
    vye = (eta * vy).astype(ml_dtypes.bfloat16)
    vze = (eta * vz).astype(ml_dtypes.bfloat16)

    idents = _idents_cache()

    arrs = {"pxm": pxm, "yg": yg, "zg": zg, "vx": vx, "vy": vy, "vz": vz,
            "mg": m, "mg16": m.astype(ml_dtypes.bfloat16),
            "vxe": vxe, "vye": vye, "vze": vze,
            "pxm16": pxm16, "yg16": yg16, "zg16": zg16}
    in_maps = []
    for c in range(N_CORES):
        sl = slice(c * ZOWN, c * ZOWN + ZS)
        im = {k: np.ascontiguousarray(a[sl].transpose(1, 0, 2))  # [YP, ZS, XP]
              for k, a in arrs.items()}
        im["idents"] = idents
        in_maps.append(im)
    return in_maps


def build_nc():
    nc = bacc.Bacc("TRN2", target_bir_lowering=False, debug=False,
                   num_devices=N_CORES)
    # const APs for ACT biases (init-time only)
    for val in (-KN, 1e-30):
        t = nc.alloc_sbuf_tensor(f"const-float32-{val}", [128, 1], F32)
        nc.gpsimd.memset(t.ap(), val)
        nc.const_aps.aps[(F32, val)] = t.ap()
    nc.all_engine_barrier()
    d = {}
    for k in POS + ["vx", "vy", "vz"]:
        d[k] = nc.dram_tensor(k, [YP, ZS, XP], F32, kind="ExternalInput")
    for k in VELE + ["mg16"]:
        d[k] = nc.dram_tensor(k, [YP, ZS, XP], BF16, kind="ExternalInput")
    for k in ["pxm16", "yg16", "zg16"]:
        d[k] = nc.dram_tensor(k, [YP, ZS, XP], FP16, kind="ExternalInput")
    idents_d = nc.dram_tensor("idents", [6, 128, 128], BF16, kind="ExternalInput")
    out_d = nc.dram_tensor("out", [6, Y, ZOWN, X], F32, kind="ExternalOutput")

    XL, XH = 2, 134            # fixed chain x-window (all ops, width 132)
    _rc = RECIP_APPROX_FAST_CONSTS

    with TileContext(nc) as tc:
        with tc.tile_pool(name="const", bufs=1) as cpool, \
             tc.tile_pool(name="cent", bufs=1) as gpool, \
             tc.tile_pool(name="shift", bufs=2) as spool, \
             tc.tile_pool(name="pvar", bufs=1) as pvpool, \
             tc.tile_pool(name="vel", bufs=2) as vpool, \
             tc.tile_pool(name="tmp", bufs=2) as tpool, \
             tc.tile_pool(name="tmp2", bufs=2) as tpool2, \
             tc.tile_pool(name="poolout", bufs=2) as plpool, \
             tc.tile_pool(name="tmpB", bufs=2) as tbpool, \
             tc.tile_pool(name="stg", bufs=1) as stpool, \
             tc.tile_pool(name="psum", bufs=1, space="PSUM") as ppool:

            idt = cpool.tile([128, 6, 128], BF16, tag="idents")
            for j in range(6):
                nc.sync.dma_start(out=idt[:, j, :], in_=idents_d[j])

            for b in range(NB):
                # center tiles: rows [2,130), z [8b, 8b+12)
                C = {}
                for k in ["pxm16", "yg16", "zg16"]:
                    t = gpool.tile([128, 12, XP], FP16, tag=f"c_{k}")
                    nc.sync.dma_start(out=t[:], in_=d[k][2:130, 8 * b:8 * b + 12, :])
                    C[k] = t
                t = gpool.tile([128, 8, XP], BF16, tag="c_mg")
                nc.sync.dma_start(out=t[:],
                                  in_=d["mg16"][2:130, 8 * b + 2:8 * b + 10, :])
                C["mg"] = t
                for k in VELE:
                    t = gpool.tile([128, 12, XP], BF16, tag=f"c_{k}")
                    nc.sync.dma_start(out=t[:], in_=d[k][2:130, 8 * b:8 * b + 12, :])
                    C[k] = t

                FX = ppool.tile([128, BS, X], F32, tag="fx")
                FY = ppool.tile([128, BS, X], F32, tag="fy")
                FZ = ppool.tile([128, BS, X], F32, tag="fz")
                started = set()
                n_done = 0

                for dy in DY_ORDER:
                    # fp16 position neighbors: staged for dy!=0, else views of
                    # the fp16 center tiles; per-group variant ops bake the d*CS
                    # constants into neighbor variants (odd dx pre-shifted 1
                    # elem for 4B alignment). Variant adds run on ACT (Copy
                    # with float bias) to keep DVE free.
                    if dy == 0:
                        S16 = {k: C[k][:, 2:12, :]
                               for k in ["pxm16", "yg16", "zg16"]}
                    else:
                        S16 = {}
                        for k in ["pxm16", "yg16", "zg16"]:
                            t = spool.tile([128, 10, XP], FP16, tag=f"s_{k}")
                            nc.sync.dma_start(
                                out=t[:],
                                in_=d[k][2 + dy:130 + dy, 2 + 8 * b:12 + 8 * b, :])
                            S16[k] = t[:]
                    PV = {}
                    for dx in range(-2, 3):
                        if dx == 0:
                            PV[0] = S16["pxm16"]
                            continue
                        pv = pvpool.tile([128, 10, XP], FP16, tag=f"pv{dx}")
                        nc.scalar.activation(pv[:], S16["pxm16"], Act.Copy,
                                             bias=float(dx * CS))
                        PV[dx] = pv
                    # even-parity y/z variants (read by even-dx offsets)
                    if dy == 0:
                        yv = S16["yg16"]
                    else:
                        yvt = pvpool.tile([128, 10, XP], FP16, tag="yv")
                        nc.scalar.activation(yvt[:], S16["yg16"], Act.Copy,
                                             bias=float(dy * CS))
                        yv = yvt[:]
                    ZV = {}
                    for dz in sorted({o[0] for o in DY_GROUPS[dy]}):
                        if dz == 0:
                            ZV[0] = S16["zg16"]
                        else:
                            zv = pvpool.tile([128, 10, XP], FP16, tag=f"zv{dz}")
                            nc.scalar.activation(zv[:], S16["zg16"], Act.Copy,
                                                 bias=float(dz * CS))
                            ZV[dz] = zv[:]
                    # velocity neighbors (bf16): staged for every dy
                    # (2B-aligned odd-dx windows read directly; HW-validated)
                    SVE = {}
                    for k in VELE:
                        te = vpool.tile([128, 10, XP], BF16, tag=f"se_{k}")
                        nc.sync.dma_start(
                            out=te[:],
                            in_=d[k][2 + dy:130 + dy, 2 + 8 * b:12 + 8 * b, :])
                        SVE[k] = te

                    for (dz, _dy, dx) in DY_GROUPS[dy]:
                        nz = BS + dz
                        zc = 2 - dz
                        # minimal x-window: union of direct [4:132) and
                        # scatter [4-dx:132-dx) consumer reads
                        XLo = 2 if dx > 0 else 4
                        XHo = XLo + (128 if dx == 0 else 130)

                        def cw(t):      # center window
                            return t[:, zc:zc + nz, XLo:XHo]

                        def nwv(k):     # velocity-neighbor window
                            return SVE[k][:, 0:nz, XLo + dx:XHo + dx]

                        def tmp(tag, dt=BF16):
                            t = tpool.tile([128, 10, XP], dt, tag=tag)
                            return t

                        def tmp2(tag, dt=BF16):
                            t = tpool2.tile([128, 10, XP], dt, tag=tag)
                            return t

                        def tmpP(tag, dt=BF16):
                            t = plpool.tile([128, 10, XP], dt, tag=tag)
                            return t

                        def tmpB(tag, dt=BF16):
                            t = tbpool.tile([128, 10, XP], dt, tag=tag)
                            return t

                        def win(t):
                            return t[:, 0:nz, XLo:XHo]

                        nby = yv[:, 0:nz, XLo + dx:XHo + dx]
                        nbz = ZV[dz][:, 0:nz, XLo + dx:XHo + dx]
                        dX = tmp2("dX"); dY_ = tmp2("dY"); dZ = tmp2("dZ")
                        nc.vector.tensor_sub(
                            win(dX), C["pxm16"][:, zc:zc + nz, XLo:XHo],
                            PV[dx][:, 0:nz, XLo + dx:XHo + dx])
                        nc.vector.tensor_sub(
                            win(dY_), C["yg16"][:, zc:zc + nz, XLo:XHo], nby)
                        nc.vector.tensor_sub(
                            win(dZ), C["zg16"][:, zc:zc + nz, XLo:XHo], nbz)

                        # r2 = dX^2+dY^2+dZ^2 (squares on ACT, adds on DVE)
                        x2 = tmpB("x2"); y2 = tmpB("y2"); z2 = tmp("z2")
                        nc.scalar.activation(win(x2), win(dX), Act.Square)
                        nc.scalar.activation(win(y2), win(dY_), Act.Square)
                        nc.scalar.activation(win(z2), win(dZ), Act.Square)
                        r2a = tmp("r2a")
                        nc.vector.tensor_add(win(r2a), win(x2), win(y2))
                        r2 = tmp("r2")
                        nc.vector.tensor_add(win(r2), win(r2a), win(z2))
                        # invK = KNCS/r in one ACT op: Rsqrt(r2/SCL + eps)
                        # (eps keeps r2==0 finite; G*d is then big*0 = 0)
                        invK = tmpB("invK")
                        nc.scalar.activation(win(invK), win(r2),
                                             Act.Abs_reciprocal_sqrt,
                                             scale=float(1.0 / SCL), bias=1e-30)

                        # velocity diffs on Pool, products + partial sums on DVE
                        dvx = tmpP("dvx"); dvy = tmpP("dvy"); dvz = tmpP("dvz")
                        nc.gpsimd.tensor_tensor(win(dvx), cw(C["vxe"]),
                                                nwv("vxe"), op=Alu.subtract)
                        nc.gpsimd.tensor_tensor(win(dvy), cw(C["vye"]),
                                                nwv("vye"), op=Alu.subtract)
                        nc.gpsimd.tensor_tensor(win(dvz), cw(C["vze"]),
                                                nwv("vze"), op=Alu.subtract)
                        vp1 = tmp2("vp1")
                        nc.vector.tensor_mul(win(vp1), win(dvx), win(dX))
                        dv = tmp("dv")
                        nc.vector.tensor_mul(win(dv), win(dvy), win(dY_))
                        nc.vector.tensor_add(win(vp1), win(vp1), win(dv))
                        dv2 = tmp("dv2")
                        nc.vector.tensor_mul(win(dv2), win(dvz), win(dZ))
                        nc.vector.tensor_add(win(vp1), win(vp1), win(dv2))
                        # G = relu(invK-KN) - (vd*invK^2/SCL)*(invK>KN)
                        G = tmp("G")
                        nc.vector._custom_dve(GFUSE, out=win(G),
                                              in0=win(vp1), in1=win(invK),
                                              s0=KN, s1=float(1.0 / SCL))

                        tX = tmpB("tX"); tY = tmpB("tY"); tZ = tmpB("tZ")
                        nc.vector.tensor_mul(win(tX), win(G), win(dX))
                        nc.vector.tensor_mul(win(tY), win(G), win(dY_))
                        nc.gpsimd.tensor_tensor(win(tZ), win(G), win(dZ),
                                                op=Alu.mult)

                        last = n_done == N_OFFS - 1
                        for (F, t) in ((FX, tX), (FY, tY), (FZ, tZ)):
                            for half in (0, 1):
                                zr = 4 * half
                                key = (id(F), half)
                                st = key not in started
                                started.add(key)
                                # direct: F[y,z',x'] += t[y, z'+dz, x'+4]
                                nc.tensor.matmul(
                                    F[:, zr:zr + 4, :],
                                    idt[:, 0, :],
                                    t[:, dz + zr:dz + zr + 4, 4:132],
                                    start=st, stop=False)
                                # scatter: F[y,z',x'] -= t[y-dy, z', x'+4-dx]
                                nc.tensor.matmul(
                                    F[:, zr:zr + 4, :],
                                    idt[:, 1 + (dy + 2), :],
                                    t[:, zr:zr + 4, 4 - dx:132 - dx],
                                    start=False, stop=last)
                        n_done += 1

                # ---- integration (pos/vel f32 loaded on demand) ----
                zv_ = slice(0, 8)
                xs = slice(4, 4 + X)
                mw = C["mg"][:, zv_, xs]
                for ci, (F, vname, pname) in enumerate(
                        ((FX, "vx", "pxm"), (FY, "vy", "yg"), (FZ, "vz", "zg"))):
                    vt = stpool.tile([128, BS, X], F32, tag="vt")
                    nc.sync.dma_start(
                        out=vt[:],
                        in_=d[vname][2:130, 8 * b + 2:8 * b + 10, 4:132])
                    pt = stpool.tile([128, BS, X], F32, tag="pt")
                    nc.sync.dma_start(
                        out=pt[:],
                        in_=d[pname][2:130, 8 * b + 2:8 * b + 10, 4:132])
                    scr = stpool.tile([128, BS, X], F32, tag="scr")
                    nc.vector.scalar_tensor_tensor(
                        scr[:], F[:], C1, vt[:],
                        op0=Alu.mult, op1=Alu.add)
                    stv = stpool.tile([128, BS, X], F32, tag="stg")
                    if ci == 2:
                        nc.vector.scalar_tensor_tensor(
                            stv[:], scr[:], GRAV * DT, mw,
                            op0=Alu.subtract, op1=Alu.mult)
                    else:
                        nc.vector.tensor_mul(stv[:], scr[:], mw)
                    nc.sync.dma_start(out=out_d[3 + ci, :, 8 * b:8 * b + BS, :],
                                      in_=stv[:])
                    scr2 = stpool.tile([128, BS, X], F32, tag="scr")
                    nc.vector.scalar_tensor_tensor(
                        scr2[:], stv[:], DT, pt[:],
                        op0=Alu.mult, op1=Alu.add)
                    stp = stpool.tile([128, BS, X], F32, tag="stg")
                    nc.vector.tensor_mul(stp[:], scr2[:], mw)
                    nc.sync.dma_start(out=out_d[ci, :, 8 * b:8 * b + BS, :],
                                      in_=stp[:])
    nc.compile()
    return nc


_NC_CACHE = []


def get_nc():
    if not _NC_CACHE:
        _NC_CACHE.append(build_nc())
    return _NC_CACHE[0]


def assemble(core_outs):
    full = np.concatenate([o.transpose(0, 2, 1, 3) for o in core_outs], axis=1)
    return full.reshape(6, 1, 1, NZ_G, Y, X)


def kernel(**inputs):
    nc = get_nc()
    in_maps = host_prepare(inputs)
    res = run_bass_kernel_spmd(nc, in_maps, core_ids=list(range(N_CORES)))
    return assemble([res.results[c]["out"] for c in range(N_CORES)])


if __name__ == "__main__":
    rng = np.random.default_rng(0)
    shp = (1, 1, NZ_G, Y, X)
    inputs = {
        "x_grid": rng.random(shp, dtype=np.float32),
        "y_grid": rng.random(shp, dtype=np.float32),
        "z_grid": rng.random(shp, dtype=np.float32),
        "vx_grid": rng.standard_normal(shp, dtype=np.float32),
        "vy_grid": rng.standard_normal(shp, dtype=np.float32),
        "vz_grid": rng.standard_normal(shp, dtype=np.float32),
        "mask_grid": np.ones(shp, dtype=np.float32),
    }
    out = kernel(**inputs)
    print("out shape:", out.shape, "finite:", np.isfinite(out).all())



# revision 16
# speedup vs baseline: 2.2223x; 1.7027x over previous
"""AI4DEM contact-force stencil kernel for 8 Trainium2 NeuronCores — v2.

v2 over the staged baseline: engine rebalance of the 62-offset pair chain.
- DVE kept for: position diffs, q products, vd partial sums, recip custom,
  new fused WGATE custom ((vd*inv2)*(inv2>1/CS^2)) and the t=G*d muls.
- ACT absorbs: the three Squares, Sqrt, a Relu-gated spring
  Sg = Relu(KNCS/r - KN) (exact: active <=> r<CS), and the neighbor-variant
  constant adds (Copy with float bias).
- Pool (gpsimd) absorbs: r2 partial add + the three velocity-diff subs.
- PE unchanged: direct+scatter force accumulation via (shifted-)identity
  matmuls into PSUM.
Gating restructure (exact algebra, no Sign/affine/+KN ops):
  G = Relu(invK - KN) - (vd*inv2)*(inv2 > 1/CS2)
"""
import math
import numpy as np
import ml_dtypes

import concourse.bass as bass
import concourse.mybir as mybir
from concourse import bacc, dve_ops
from concourse.dve_spec import Spec, Src0, Src1, C0, lower, _has_src1, relu, sq
from concourse.dve_spec import C1 as DC1
from concourse.dve_uop import DveOpSpec
from concourse.tile import TileContext
from concourse.bass_utils import run_bass_kernel_spmd
from concourse.dve_ops import RECIP_APPROX_FAST_CONSTS, RECIPROCAL_APPROX_FAST

Alu = mybir.AluOpType
Act = mybir.ActivationFunctionType
F32 = mybir.dt.float32
BF16 = mybir.dt.bfloat16
FP16 = mybir.dt.float16

# --- physics constants (match reference) ---
CS = 0.05
KN = 600000.0
DT = 0.001
GRAV = 9.8
_ALPHA = -math.log(0.5) / math.pi
_GAMMA = _ALPHA / math.sqrt(_ALPHA**2 + 1.0)
MASS = 4.0 / 3.0 * 3.1416 * CS**3 * 2700.0
ETA = 2.0 * _GAMMA * math.sqrt(KN * MASS / 2.0)
KNCS = KN * CS
SCL = KNCS * KNCS          # ACT Sqrt scale: sqrt(SCL*inv2) = KNCS/r
CS2 = float(np.float32(CS) * np.float32(CS))
INV_CS2 = float(1.0 / CS2)
C1 = DT / MASS

NZ_G, Y, X = 128, 128, 128
XP, YP, ZS, ZOWN = 136, 132, 20, 16
NB, BS = 2, 8
N_CORES = 8

OFFS = [(dz, dy, dx)
        for dz in range(0, 3) for dy in range(-2, 3) for dx in range(-2, 3)
        if (dz > 0) or (dz == 0 and dy > 0) or (dz == 0 and dy == 0 and dx > 0)]
DY_ORDER = [0, 1, 2, -1, -2]
DY_GROUPS = {dy: [o for o in OFFS if o[1] == dy] for dy in DY_ORDER}
N_OFFS = len(OFFS)
POS = ["pxm", "yg", "zg"]
VELE = ["vxe", "vye", "vze"]


def _register_dve_op(name, spec, subdim=False):
    if name in dve_ops._SUB_OPCODE_FOR_NAME:
        return next(o for o in dve_ops.OPS if o.name == name)
    row = dve_ops._CUSTOM_DVE_ROW_BASE + len(dve_ops.OPS)
    assert row < 0x20
    shas = {}
    for ver in ("v3", "v4"):
        try:
            uops = lower(spec, ver=ver)
            shas[ver] = DveOpSpec(name=name, opcode=row, uops=uops,
                                  rd1_en=_has_src1(spec)).sha(ver)
        except ValueError:
            pass
    op = dve_ops.DveOp(name, spec, subdim=subdim, uops_sha=shas)
    dve_ops.OPS.append(op)
    dve_ops.CUSTOM_DVE_SPECS[name] = spec
    dve_ops._SUB_OPCODE_FOR_NAME[name] = row
    return op


# wg = (vd * inv2) * (inv2 > 1/CS2): gated damping term in one DVE inst
WGATE = _register_dve_op(
    "WGATE_AI4",
    Spec(
        body=(Src0 * Src1) * (Src1 > C0),
        reference=lambda in0, in1, s0, s1, imm2: (
            (in0.astype(np.float32) * in1) * (in1 > s0)
        ).astype(np.float32),
    ),
)


# G = relu(invK - KN) - ((vd * invK^2) * (1/SCL)) * (invK > KN)
# (one inst for spring + gated damping + combine; gate invK>KN == r<CS)
GFUSE = _register_dve_op(
    "GFUSE_AI4",
    Spec(
        body=relu(Src1 - C0) - ((Src0 * sq(Src1)) * DC1) * (Src1 > C0),
        reference=lambda in0, in1, s0, s1, imm2: (
            np.maximum(in1.astype(np.float32) - s0, 0)
            - ((in0.astype(np.float32) * in1.astype(np.float32) ** 2) * s1)
            * (in1 > s0)
        ).astype(np.float32),
    ),
)


_HOST_CONST = {}


def _rmod_cache():
    if "rmod" not in _HOST_CONST:
        zi, yi, xi = np.meshgrid(np.arange(NZ_G + 4), np.arange(YP),
                                 np.arange(XP), indexing="ij")
        _HOST_CONST["rmod"] = ((zi % 5) * 25 + (yi % 5) * 5 + (xi % 5)
                               + 1).astype(np.float32)
    return _HOST_CONST["rmod"]


def _idents_cache():
    if "idents" not in _HOST_CONST:
        _HOST_CONST["idents"] = np.stack(
            [np.eye(128, dtype=np.float32)] +
            [-np.eye(128, k=dy, dtype=np.float32) for dy in range(-2, 3)]
        ).astype(ml_dtypes.bfloat16)
    return _HOST_CONST["idents"]


def host_prepare(inputs):
    """Full inputs -> list of 8 per-core in_maps."""
    full = {k: np.asarray(v, dtype=np.float32).reshape(NZ_G, Y, X)
            for k, v in inputs.items()}

    def pad(a):
        return np.pad(a, ((2, 2), (2, 2), (4, 4)))

    xg = pad(full["x_grid"]); yg = pad(full["y_grid"]); zg = pad(full["z_grid"])
    vx = pad(full["vx_grid"]); vy = pad(full["vy_grid"]); vz = pad(full["vz_grid"])
    m = pad(full["mask_grid"])
    rmod = _rmod_cache()
    half = np.float32(0.5)
    disp = np.float32(5.0) * (np.float32(1.0) - m) * rmod
    pxm16 = ((xg - half) + disp).astype(np.float16)
    yg16 = (yg - half).astype(np.float16)
    zg16 = (zg - half).astype(np.float16)
    pxm = xg
    eta = np.float32(ETA)
    vxe = (eta * vx).astype(ml_dtypes.bfloat16)
    vye = (eta * vy).astype(ml_dtypes.bfloat16)
    vze = (eta * vz).astype(ml_dtypes.bfloat16)

    idents = _idents_cache()

    arrs = {"pxm": pxm, "yg": yg, "zg": zg, "vx": vx, "vy": vy, "vz": vz,
            "mg": m, "mg16": m.astype(ml_dtypes.bfloat16),
            "vxe": vxe, "vye": vye, "vze": vze,
            "pxm16": pxm16, "yg16": yg16, "zg16": zg16}
    in_maps = []
    for c in range(N_CORES):
        sl = slice(c * ZOWN, c * ZOWN + ZS)
        im = {k: np.ascontiguousarray(a[sl].transpose(1, 0, 2))  # [YP, ZS, XP]
              for k, a in arrs.items()}
        # pack the pair-loop streams: pos3 fp16 / vel3 bf16 [YP, 3, ZS, XP]
        im["pos3"] = np.ascontiguousarray(
            np.stack([im.pop("pxm16"), im.pop("yg16"), im.pop("zg16")], axis=1))
        im["vel3"] = np.ascontiguousarray(
            np.stack([im.pop("vxe"), im.pop("vye"), im.pop("vze")], axis=1))
        im["idents"] = idents
        in_maps.append(im)
    return in_maps


def build_nc():
    nc = bacc.Bacc("TRN2", target_bir_lowering=False, debug=False,
                   num_devices=N_CORES)
    # const APs for ACT biases (init-time only)
    for val in (-KN, 1e-30):
        t = nc.alloc_sbuf_tensor(f"const-float32-{val}", [128, 1], F32)
        nc.gpsimd.memset(t.ap(), val)
        nc.const_aps.aps[(F32, val)] = t.ap()
    nc.all_engine_barrier()
    d = {}
    for k in POS + ["vx", "vy", "vz"]:
        d[k] = nc.dram_tensor(k, [YP, ZS, XP], F32, kind="ExternalInput")
    d["mg16"] = nc.dram_tensor("mg16", [YP, ZS, XP], BF16, kind="ExternalInput")
    d["pos3"] = nc.dram_tensor("pos3", [YP, 3, ZS, XP], FP16,
                               kind="ExternalInput")
    d["vel3"] = nc.dram_tensor("vel3", [YP, 3, ZS, XP], BF16,
                               kind="ExternalInput")
    idents_d = nc.dram_tensor("idents", [6, 128, 128], BF16, kind="ExternalInput")
    out_d = nc.dram_tensor("out", [6, Y, ZOWN, X], F32, kind="ExternalOutput")

    XL, XH = 2, 134            # fixed chain x-window (all ops, width 132)
    _rc = RECIP_APPROX_FAST_CONSTS

    with TileContext(nc) as tc:
        with tc.tile_pool(name="const", bufs=1) as cpool, \
             tc.tile_pool(name="cent", bufs=1) as gpool, \
             tc.tile_pool(name="shift", bufs=2) as spool, \
             tc.tile_pool(name="pvar", bufs=1) as pvpool, \
             tc.tile_pool(name="vel", bufs=2) as vpool, \
             tc.tile_pool(name="tmp", bufs=2) as tpool, \
             tc.tile_pool(name="tmp2", bufs=2) as tpool2, \
             tc.tile_pool(name="poolout", bufs=2) as plpool, \
             tc.tile_pool(name="tmpB", bufs=2) as tbpool, \
             tc.tile_pool(name="stg", bufs=1) as stpool, \
             tc.tile_pool(name="psum", bufs=1, space="PSUM") as ppool:

            idt = cpool.tile([128, 6, 128], BF16, tag="idents")
            for j in range(6):
                nc.sync.dma_start(out=idt[:, j, :], in_=idents_d[j])

            for b in range(NB):
                # center tiles: rows [2,130), z [8b, 8b+12); packed 3-field
                # loads (one DMA each for pos3/vel3)
                C = {}
                cpos = gpool.tile([128, 3, 12, XP], FP16, tag="c_pos3")
                nc.sync.dma_start(out=cpos[:],
                                  in_=d["pos3"][2:130, :, 8 * b:8 * b + 12, :])
                for j, k in enumerate(["pxm16", "yg16", "zg16"]):
                    C[k] = cpos[:, j]
                t = gpool.tile([128, 8, XP], BF16, tag="c_mg")
                nc.sync.dma_start(out=t[:],
                                  in_=d["mg16"][2:130, 8 * b + 2:8 * b + 10, :])
                C["mg"] = t
                cvel = gpool.tile([128, 3, 12, XP], BF16, tag="c_vel3")
                nc.sync.dma_start(out=cvel[:],
                                  in_=d["vel3"][2:130, :, 8 * b:8 * b + 12, :])
                for j, k in enumerate(VELE):
                    C[k] = cvel[:, j]

                FX = ppool.tile([128, BS, X], F32, tag="fx")
                FY = ppool.tile([128, BS, X], F32, tag="fy")
                FZ = ppool.tile([128, BS, X], F32, tag="fz")
                started = set()
                n_done = 0

                for dy in DY_ORDER:
                    # fp16 position neighbors: staged for dy!=0, else views of
                    # the fp16 center tiles; per-group variant ops bake the d*CS
                    # constants into neighbor variants (odd dx pre-shifted 1
                    # elem for 4B alignment). Variant adds run on ACT (Copy
                    # with float bias) to keep DVE free.
                    if dy == 0:
                        S16 = {k: C[k][:, 2:12, :]
                               for k in ["pxm16", "yg16", "zg16"]}
                    else:
                        st = spool.tile([128, 3, 10, XP], FP16, tag="s_pos3")
                        nc.sync.dma_start(
                            out=st[:],
                            in_=d["pos3"][2 + dy:130 + dy, :,
                                          2 + 8 * b:12 + 8 * b, :])
                        S16 = {k: st[:, j]
                               for j, k in enumerate(["pxm16", "yg16", "zg16"])}
                    PV = {}
                    for dx in range(-2, 3):
                        if dx == 0:
                            PV[0] = S16["pxm16"]
                            continue
                        pv = pvpool.tile([128, 10, XP], FP16, tag=f"pv{dx}")
                        nc.scalar.activation(pv[:], S16["pxm16"], Act.Copy,
                                             bias=float(dx * CS))
                        PV[dx] = pv
                    # even-parity y/z variants (read by even-dx offsets)
                    if dy == 0:
                        yv = S16["yg16"]
                    else:
                        yvt = pvpool.tile([128, 10, XP], FP16, tag="yv")
                        nc.scalar.activation(yvt[:], S16["yg16"], Act.Copy,
                                             bias=float(dy * CS))
                        yv = yvt[:]
                    ZV = {}
                    for dz in sorted({o[0] for o in DY_GROUPS[dy]}):
                        if dz == 0:
                            ZV[0] = S16["zg16"]
                        else:
                            zv = pvpool.tile([128, 10, XP], FP16, tag=f"zv{dz}")
                            nc.scalar.activation(zv[:], S16["zg16"], Act.Copy,
                                                 bias=float(dz * CS))
                            ZV[dz] = zv[:]
                    # velocity neighbors (bf16): dy=0 reuses the center vel
                    # tile (rows [2,12)); other dy staged in one packed DMA
                    if dy == 0:
                        SVE = {k: C[k][:, 2:12, :] for k in VELE}
                    else:
                        sv = vpool.tile([128, 3, 10, XP], BF16, tag="s_vel3")
                        nc.sync.dma_start(
                            out=sv[:],
                            in_=d["vel3"][2 + dy:130 + dy, :,
                                          2 + 8 * b:12 + 8 * b, :])
                        SVE = {k: sv[:, j] for j, k in enumerate(VELE)}

                    for (dz, _dy, dx) in DY_GROUPS[dy]:
                        nz = BS + dz
                        zc = 2 - dz
                        # minimal x-window: union of direct [4:132) and
                        # scatter [4-dx:132-dx) consumer reads
                        XLo = 2 if dx > 0 else 4
                        XHo = XLo + (128 if dx == 0 else 130)

                        def cw(t):      # center window
                            return t[:, zc:zc + nz, XLo:XHo]

                        def nwv(k):     # velocity-neighbor window
                            return SVE[k][:, 0:nz, XLo + dx:XHo + dx]

                        def tmp(tag, dt=BF16):
                            t = tpool.tile([128, 10, XP], dt, tag=tag)
                            return t

                        def tmp2(tag, dt=BF16):
                            t = tpool2.tile([128, 10, XP], dt, tag=tag)
                            return t

                        def tmpP(tag, dt=BF16):
                            t = plpool.tile([128, 10, XP], dt, tag=tag)
                            return t

                        def tmpB(tag, dt=BF16):
                            t = tbpool.tile([128, 10, XP], dt, tag=tag)
                            return t

                        def win(t):
                            return t[:, 0:nz, XLo:XHo]

                        nby = yv[:, 0:nz, XLo + dx:XHo + dx]
                        nbz = ZV[dz][:, 0:nz, XLo + dx:XHo + dx]
                        dX = tmp2("dX"); dY_ = tmp2("dY"); dZ = tmp2("dZ")
                        nc.vector.tensor_sub(
                            win(dX), C["pxm16"][:, zc:zc + nz, XLo:XHo],
                            PV[dx][:, 0:nz, XLo + dx:XHo + dx])
                        nc.vector.tensor_sub(
                            win(dY_), C["yg16"][:, zc:zc + nz, XLo:XHo], nby)
                        nc.vector.tensor_sub(
                            win(dZ), C["zg16"][:, zc:zc + nz, XLo:XHo], nbz)

                        # r2 = dX^2+dY^2+dZ^2 (squares on ACT, adds on DVE)
                        x2 = tmpB("x2"); y2 = tmpB("y2"); z2 = tmp("z2")
                        nc.scalar.activation(win(x2), win(dX), Act.Square)
                        nc.scalar.activation(win(y2), win(dY_), Act.Square)
                        nc.scalar.activation(win(z2), win(dZ), Act.Square)
                        r2a = tmp("r2a")
                        nc.vector.tensor_add(win(r2a), win(x2), win(y2))
                        r2 = tmp("r2")
                        nc.vector.tensor_add(win(r2), win(r2a), win(z2))
                        # invK = KNCS/r in one ACT op: Rsqrt(r2/SCL + eps)
                        # (eps keeps r2==0 finite; G*d is then big*0 = 0)
                        invK = tmpB("invK")
                        nc.scalar.activation(win(invK), win(r2),
                                             Act.Abs_reciprocal_sqrt,
                                             scale=float(1.0 / SCL), bias=1e-30)

                        # velocity diffs on Pool, products + partial sums on DVE
                        dvx = tmpP("dvx"); dvy = tmpP("dvy"); dvz = tmpP("dvz")
                        nc.gpsimd.tensor_tensor(win(dvx), cw(C["vxe"]),
                                                nwv("vxe"), op=Alu.subtract)
                        nc.gpsimd.tensor_tensor(win(dvy), cw(C["vye"]),
                                                nwv("vye"), op=Alu.subtract)
                        nc.gpsimd.tensor_tensor(win(dvz), cw(C["vze"]),
                                                nwv("vze"), op=Alu.subtract)
                        vp1 = tmp2("vp1")
                        nc.vector.tensor_mul(win(vp1), win(dvx), win(dX))
                        dv = tmp("dv")
                        nc.vector.tensor_mul(win(dv), win(dvy), win(dY_))
                        nc.vector.tensor_add(win(vp1), win(vp1), win(dv))
                        dv2 = tmp("dv2")
                        nc.vector.tensor_mul(win(dv2), win(dvz), win(dZ))
                        nc.vector.tensor_add(win(vp1), win(vp1), win(dv2))
                        # G = relu(invK-KN) - (vd*invK^2/SCL)*(invK>KN)
                        G = tmp("G")
                        nc.vector._custom_dve(GFUSE, out=win(G),
                                              in0=win(vp1), in1=win(invK),
                                              s0=KN, s1=float(1.0 / SCL))

                        tX = tmpB("tX"); tY = tmpB("tY"); tZ = tmpB("tZ")
                        nc.vector.tensor_mul(win(tX), win(G), win(dX))
                        nc.vector.tensor_mul(win(tY), win(G), win(dY_))
                        nc.gpsimd.tensor_tensor(win(tZ), win(G), win(dZ),
                                                op=Alu.mult)

                        last = n_done == N_OFFS - 1
                        for (F, t) in ((FX, tX), (FY, tY), (FZ, tZ)):
                            for half in (0, 1):
                                zr = 4 * half
                                key = (id(F), half)
                                st = key not in started
                                started.add(key)
                                # direct: F[y,z',x'] += t[y, z'+dz, x'+4]
                                nc.tensor.matmul(
                                    F[:, zr:zr + 4, :],
                                    idt[:, 0, :],
                                    t[:, dz + zr:dz + zr + 4, 4:132],
                                    start=st, stop=False)
                                # scatter: F[y,z',x'] -= t[y-dy, z', x'+4-dx]
                                nc.tensor.matmul(
                                    F[:, zr:zr + 4, :],
                                    idt[:, 1 + (dy + 2), :],
                                    t[:, zr:zr + 4, 4 - dx:132 - dx],
                                    start=False, stop=last)
                        n_done += 1

                # ---- integration (pos/vel f32 loaded on demand) ----
                zv_ = slice(0, 8)
                xs = slice(4, 4 + X)
                mw = C["mg"][:, zv_, xs]
                for ci, (F, vname, pname) in enumerate(
                        ((FX, "vx", "pxm"), (FY, "vy", "yg"), (FZ, "vz", "zg"))):
                    vt = stpool.tile([128, BS, X], F32, tag="vt")
                    nc.sync.dma_start(
                        out=vt[:],
                        in_=d[vname][2:130, 8 * b + 2:8 * b + 10, 4:132])
                    pt = stpool.tile([128, BS, X], F32, tag="pt")
                    nc.sync.dma_start(
                        out=pt[:],
                        in_=d[pname][2:130, 8 * b + 2:8 * b + 10, 4:132])
                    scr = stpool.tile([128, BS, X], F32, tag="scr")
                    nc.vector.scalar_tensor_tensor(
                        scr[:], F[:], C1, vt[:],
                        op0=Alu.mult, op1=Alu.add)
                    stv = stpool.tile([128, BS, X], F32, tag="stg")
                    if ci == 2:
                        nc.vector.scalar_tensor_tensor(
                            stv[:], scr[:], GRAV * DT, mw,
                            op0=Alu.subtract, op1=Alu.mult)
                    else:
                        nc.vector.tensor_mul(stv[:], scr[:], mw)
                    nc.sync.dma_start(out=out_d[3 + ci, :, 8 * b:8 * b + BS, :],
                                      in_=stv[:])
                    scr2 = stpool.tile([128, BS, X], F32, tag="scr")
                    nc.vector.scalar_tensor_tensor(
                        scr2[:], stv[:], DT, pt[:],
                        op0=Alu.mult, op1=Alu.add)
                    stp = stpool.tile([128, BS, X], F32, tag="stg")
                    nc.vector.tensor_mul(stp[:], scr2[:], mw)
                    nc.sync.dma_start(out=out_d[ci, :, 8 * b:8 * b + BS, :],
                                      in_=stp[:])
    nc.compile()
    return nc


_NC_CACHE = []


def get_nc():
    if not _NC_CACHE:
        _NC_CACHE.append(build_nc())
    return _NC_CACHE[0]


def assemble(core_outs):
    full = np.concatenate([o.transpose(0, 2, 1, 3) for o in core_outs], axis=1)
    return full.reshape(6, 1, 1, NZ_G, Y, X)


def kernel(**inputs):
    nc = get_nc()
    in_maps = host_prepare(inputs)
    res = run_bass_kernel_spmd(nc, in_maps, core_ids=list(range(N_CORES)))
    return assemble([res.results[c]["out"] for c in range(N_CORES)])


if __name__ == "__main__":
    rng = np.random.default_rng(0)
    shp = (1, 1, NZ_G, Y, X)
    inputs = {
        "x_grid": rng.random(shp, dtype=np.float32),
        "y_grid": rng.random(shp, dtype=np.float32),
        "z_grid": rng.random(shp, dtype=np.float32),
        "vx_grid": rng.standard_normal(shp, dtype=np.float32),
        "vy_grid": rng.standard_normal(shp, dtype=np.float32),
        "vz_grid": rng.standard_normal(shp, dtype=np.float32),
        "mask_grid": np.ones(shp, dtype=np.float32),
    }
    out = kernel(**inputs)
    print("out shape:", out.shape, "finite:", np.isfinite(out).all())



# revision 18
# speedup vs baseline: 6.2790x; 2.8254x over previous
"""AI4DEM contact-force stencil kernel for 8 Trainium2 NeuronCores — v2.

v2 over the staged baseline: engine rebalance of the 62-offset pair chain.
- DVE kept for: position diffs, q products, vd partial sums, recip custom,
  new fused WGATE custom ((vd*inv2)*(inv2>1/CS^2)) and the t=G*d muls.
- ACT absorbs: the three Squares, Sqrt, a Relu-gated spring
  Sg = Relu(KNCS/r - KN) (exact: active <=> r<CS), and the neighbor-variant
  constant adds (Copy with float bias).
- Pool (gpsimd) absorbs: r2 partial add + the three velocity-diff subs.
- PE unchanged: direct+scatter force accumulation via (shifted-)identity
  matmuls into PSUM.
Gating restructure (exact algebra, no Sign/affine/+KN ops):
  G = Relu(invK - KN) - (vd*inv2)*(inv2 > 1/CS2)
"""
import math
import numpy as np
import ml_dtypes

import concourse.bass as bass
import concourse.mybir as mybir
from concourse import bacc, dve_ops
from concourse.dve_spec import Spec, Src0, Src1, C0, lower, _has_src1, relu, sq
from concourse.dve_spec import C1 as DC1
from concourse.dve_uop import DveOpSpec
from concourse.tile import TileContext
from concourse.bass_utils import run_bass_kernel_spmd
from concourse.dve_ops import RECIP_APPROX_FAST_CONSTS, RECIPROCAL_APPROX_FAST

Alu = mybir.AluOpType
Act = mybir.ActivationFunctionType
F32 = mybir.dt.float32
BF16 = mybir.dt.bfloat16
FP16 = mybir.dt.float16

# --- physics constants (match reference) ---
CS = 0.05
KN = 600000.0
DT = 0.001
GRAV = 9.8
_ALPHA = -math.log(0.5) / math.pi
_GAMMA = _ALPHA / math.sqrt(_ALPHA**2 + 1.0)
MASS = 4.0 / 3.0 * 3.1416 * CS**3 * 2700.0
ETA = 2.0 * _GAMMA * math.sqrt(KN * MASS / 2.0)
KNCS = KN * CS
SCL = KNCS * KNCS          # ACT Sqrt scale: sqrt(SCL*inv2) = KNCS/r
CS2 = float(np.float32(CS) * np.float32(CS))
INV_CS2 = float(1.0 / CS2)
C1 = DT / MASS

NZ_G, Y, X = 128, 128, 128
XP, YP, ZS, ZOWN = 136, 132, 20, 16
NB, BS = 2, 8
N_CORES = 8

OFFS = [(dz, dy, dx)
        for dz in range(0, 3) for dy in range(-2, 3) for dx in range(-2, 3)
        if (dz > 0) or (dz == 0 and dy > 0) or (dz == 0 and dy == 0 and dx > 0)]
DY_ORDER = [0, 1, 2, -1, -2]
DY_GROUPS = {dy: [o for o in OFFS if o[1] == dy] for dy in DY_ORDER}
N_OFFS = len(OFFS)
POS = ["pxm", "yg", "zg"]
VELE = ["vxe", "vye", "vze"]


def _register_dve_op(name, spec, subdim=False):
    if name in dve_ops._SUB_OPCODE_FOR_NAME:
        return next(o for o in dve_ops.OPS if o.name == name)
    row = dve_ops._CUSTOM_DVE_ROW_BASE + len(dve_ops.OPS)
    assert row < 0x20
    shas = {}
    for ver in ("v3", "v4"):
        try:
            uops = lower(spec, ver=ver)
            shas[ver] = DveOpSpec(name=name, opcode=row, uops=uops,
                                  rd1_en=_has_src1(spec)).sha(ver)
        except ValueError:
            pass
    op = dve_ops.DveOp(name, spec, subdim=subdim, uops_sha=shas)
    dve_ops.OPS.append(op)
    dve_ops.CUSTOM_DVE_SPECS[name] = spec
    dve_ops._SUB_OPCODE_FOR_NAME[name] = row
    return op


# wg = (vd * inv2) * (inv2 > 1/CS2): gated damping term in one DVE inst
WGATE = _register_dve_op(
    "WGATE_AI4",
    Spec(
        body=(Src0 * Src1) * (Src1 > C0),
        reference=lambda in0, in1, s0, s1, imm2: (
            (in0.astype(np.float32) * in1) * (in1 > s0)
        ).astype(np.float32),
    ),
)


# G = relu(invK - KN) - ((vd * invK^2) * (1/SCL)) * (invK > KN)
# (one inst for spring + gated damping + combine; gate invK>KN == r<CS)
GFUSE = _register_dve_op(
    "GFUSE_AI4",
    Spec(
        body=relu(Src1 - C0) - ((Src0 * sq(Src1)) * DC1) * (Src1 > C0),
        reference=lambda in0, in1, s0, s1, imm2: (
            np.maximum(in1.astype(np.float32) - s0, 0)
            - ((in0.astype(np.float32) * in1.astype(np.float32) ** 2) * s1)
            * (in1 > s0)
        ).astype(np.float32),
    ),
)


_HOST_CONST = {}


def _rmod_cache():
    if "rmod" not in _HOST_CONST:
        zi, yi, xi = np.meshgrid(np.arange(NZ_G + 4), np.arange(YP),
                                 np.arange(XP), indexing="ij")
        _HOST_CONST["rmod"] = ((zi % 5) * 25 + (yi % 5) * 5 + (xi % 5)
                               + 1).astype(np.float32)
    return _HOST_CONST["rmod"]


def _idents_cache():
    if "idents" not in _HOST_CONST:
        _HOST_CONST["idents"] = np.stack(
            [np.eye(128, dtype=np.float32)] +
            [-np.eye(128, k=dy, dtype=np.float32) for dy in range(-2, 3)]
        ).astype(ml_dtypes.bfloat16)
    return _HOST_CONST["idents"]


def host_prepare(inputs):
    """Full inputs -> list of 8 per-core in_maps."""
    full = {k: np.asarray(v, dtype=np.float32).reshape(NZ_G, Y, X)
            for k, v in inputs.items()}

    def pad(a):
        return np.pad(a, ((2, 2), (2, 2), (4, 4)))

    xg = pad(full["x_grid"]); yg = pad(full["y_grid"]); zg = pad(full["z_grid"])
    vx = pad(full["vx_grid"]); vy = pad(full["vy_grid"]); vz = pad(full["vz_grid"])
    m = pad(full["mask_grid"])
    rmod = _rmod_cache()
    half = np.float32(0.5)
    disp = np.float32(5.0) * (np.float32(1.0) - m) * rmod
    pxm16 = ((xg - half) + disp).astype(np.float16)
    yg16 = (yg - half).astype(np.float16)
    zg16 = (zg - half).astype(np.float16)
    pxm = xg
    eta = np.float32(ETA)
    vxe = (eta * vx).astype(ml_dtypes.bfloat16)
    vye = (eta * vy).astype(ml_dtypes.bfloat16)
    vze = (eta * vz).astype(ml_dtypes.bfloat16)

    idents = _idents_cache()

    arrs = {"pxm": pxm, "yg": yg, "zg": zg, "vx": vx, "vy": vy, "vz": vz,
            "mg": m, "mg16": m.astype(ml_dtypes.bfloat16),
            "vxe": vxe, "vye": vye, "vze": vze,
            "pxm16": pxm16, "yg16": yg16, "zg16": zg16}
    in_maps = []
    for c in range(N_CORES):
        sl = slice(c * ZOWN, c * ZOWN + ZS)
        im = {k: np.ascontiguousarray(a[sl].transpose(1, 0, 2))  # [YP, ZS, XP]
              for k, a in arrs.items()}
        # pack the pair-loop streams: pos3 fp16 / vel3 bf16 [YP, 3, ZS, XP]
        im["pos3"] = np.ascontiguousarray(
            np.stack([im.pop("pxm16"), im.pop("yg16"), im.pop("zg16")], axis=1))
        im["vel3"] = np.ascontiguousarray(
            np.stack([im.pop("vxe"), im.pop("vye"), im.pop("vze")], axis=1))
        im["idents"] = idents
        in_maps.append(im)
    return in_maps


def build_nc():
    nc = bacc.Bacc("TRN2", target_bir_lowering=False, debug=False,
                   num_devices=N_CORES)
    # const APs for ACT biases (init-time only)
    for val in (-KN, 1e-30):
        t = nc.alloc_sbuf_tensor(f"const-float32-{val}", [128, 1], F32)
        nc.gpsimd.memset(t.ap(), val)
        nc.const_aps.aps[(F32, val)] = t.ap()
    nc.all_engine_barrier()
    d = {}
    for k in POS + ["vx", "vy", "vz"]:
        d[k] = nc.dram_tensor(k, [YP, ZS, XP], F32, kind="ExternalInput")
    d["mg16"] = nc.dram_tensor("mg16", [YP, ZS, XP], BF16, kind="ExternalInput")
    d["pos3"] = nc.dram_tensor("pos3", [YP, 3, ZS, XP], FP16,
                               kind="ExternalInput")
    d["vel3"] = nc.dram_tensor("vel3", [YP, 3, ZS, XP], BF16,
                               kind="ExternalInput")
    idents_d = nc.dram_tensor("idents", [6, 128, 128], BF16, kind="ExternalInput")
    out_d = nc.dram_tensor("out", [6, Y, ZOWN, X], F32, kind="ExternalOutput")

    XL, XH = 2, 134            # fixed chain x-window (all ops, width 132)
    _rc = RECIP_APPROX_FAST_CONSTS

    with TileContext(nc) as tc:
        with tc.tile_pool(name="const", bufs=1) as cpool, \
             tc.tile_pool(name="cent", bufs=1) as gpool, \
             tc.tile_pool(name="shift", bufs=2) as spool, \
             tc.tile_pool(name="pvar", bufs=1) as pvpool, \
             tc.tile_pool(name="vel", bufs=2) as vpool, \
             tc.tile_pool(name="tmp", bufs=2) as tpool, \
             tc.tile_pool(name="tmp2", bufs=2) as tpool2, \
             tc.tile_pool(name="poolout", bufs=2) as plpool, \
             tc.tile_pool(name="tmpB", bufs=2) as tbpool, \
             tc.tile_pool(name="stg", bufs=1) as stpool, \
             tc.tile_pool(name="psum", bufs=1, space="PSUM") as ppool:

            idt = cpool.tile([128, 6, 128], BF16, tag="idents")
            for j in range(6):
                nc.sync.dma_start(out=idt[:, j, :], in_=idents_d[j])

            for b in range(NB):
                # center tiles: rows [2,130), z [8b, 8b+12); packed 3-field
                # loads (one DMA each for pos3/vel3)
                C = {}
                cpos = gpool.tile([128, 3, 12, XP], FP16, tag="c_pos3")
                nc.sync.dma_start(out=cpos[:],
                                  in_=d["pos3"][2:130, :, 8 * b:8 * b + 12, :])
                for j, k in enumerate(["pxm16", "yg16", "zg16"]):
                    C[k] = cpos[:, j]
                t = gpool.tile([128, 8, XP], BF16, tag="c_mg")
                nc.sync.dma_start(out=t[:],
                                  in_=d["mg16"][2:130, 8 * b + 2:8 * b + 10, :])
                C["mg"] = t
                cvel = gpool.tile([128, 3, 12, XP], BF16, tag="c_vel3")
                nc.sync.dma_start(out=cvel[:],
                                  in_=d["vel3"][2:130, :, 8 * b:8 * b + 12, :])
                for j, k in enumerate(VELE):
                    C[k] = cvel[:, j]

                FX = ppool.tile([128, BS, X], F32, tag="fx")
                FY = ppool.tile([128, BS, X], F32, tag="fy")
                FZ = ppool.tile([128, BS, X], F32, tag="fz")
                started = set()
                n_done = 0

                for dy in DY_ORDER:
                    # fp16 position neighbors: staged for dy!=0, else views of
                    # the fp16 center tiles; per-group variant ops bake the d*CS
                    # constants into neighbor variants (odd dx pre-shifted 1
                    # elem for 4B alignment). Variant adds run on ACT (Copy
                    # with float bias) to keep DVE free.
                    if dy == 0:
                        S16 = {k: C[k][:, 2:12, :]
                               for k in ["pxm16", "yg16", "zg16"]}
                    else:
                        st = spool.tile([128, 3, 10, XP], FP16, tag="s_pos3")
                        nc.sync.dma_start(
                            out=st[:],
                            in_=d["pos3"][2 + dy:130 + dy, :,
                                          2 + 8 * b:12 + 8 * b, :])
                        S16 = {k: st[:, j]
                               for j, k in enumerate(["pxm16", "yg16", "zg16"])}
                    PV = {}
                    for dx in range(-2, 3):
                        if dx == 0:
                            PV[0] = S16["pxm16"]
                            continue
                        pv = pvpool.tile([128, 10, XP], FP16, tag=f"pv{dx}")
                        nc.scalar.activation(pv[:], S16["pxm16"], Act.Copy,
                                             bias=float(dx * CS))
                        PV[dx] = pv
                    # even-parity y/z variants (read by even-dx offsets)
                    if dy == 0:
                        yv = S16["yg16"]
                    else:
                        yvt = pvpool.tile([128, 10, XP], FP16, tag="yv")
                        nc.scalar.activation(yvt[:], S16["yg16"], Act.Copy,
                                             bias=float(dy * CS))
                        yv = yvt[:]
                    ZV = {}
                    for dz in sorted({o[0] for o in DY_GROUPS[dy]}):
                        if dz == 0:
                            ZV[0] = S16["zg16"]
                        else:
                            zv = pvpool.tile([128, 10, XP], FP16, tag=f"zv{dz}")
                            nc.scalar.activation(zv[:], S16["zg16"], Act.Copy,
                                                 bias=float(dz * CS))
                            ZV[dz] = zv[:]
                    # velocity neighbors (bf16): dy=0 reuses the center vel
                    # tile (rows [2,12)); other dy staged in one packed DMA
                    if dy == 0:
                        SVE = {k: C[k][:, 2:12, :] for k in VELE}
                    else:
                        sv = vpool.tile([128, 3, 10, XP], BF16, tag="s_vel3")
                        nc.sync.dma_start(
                            out=sv[:],
                            in_=d["vel3"][2 + dy:130 + dy, :,
                                          2 + 8 * b:12 + 8 * b, :])
                        SVE = {k: sv[:, j] for j, k in enumerate(VELE)}

                    for (dz, _dy, dx) in DY_GROUPS[dy]:
                        nz = BS + dz
                        zc = 2 - dz
                        # minimal x-window: union of direct [4:132) and
                        # scatter [4-dx:132-dx) consumer reads
                        XLo = 2 if dx > 0 else 4
                        XHo = XLo + (128 if dx == 0 else 130)

                        def cw(t):      # center window
                            return t[:, zc:zc + nz, XLo:XHo]

                        def nwv(k):     # velocity-neighbor window
                            return SVE[k][:, 0:nz, XLo + dx:XHo + dx]

                        def tmp(tag, dt=BF16):
                            t = tpool.tile([128, 10, XP], dt, tag=tag)
                            return t

                        def tmp2(tag, dt=BF16):
                            t = tpool2.tile([128, 10, XP], dt, tag=tag)
                            return t

                        def tmpP(tag, dt=BF16):
                            t = plpool.tile([128, 10, XP], dt, tag=tag)
                            return t

                        def tmpB(tag, dt=BF16):
                            t = tbpool.tile([128, 10, XP], dt, tag=tag)
                            return t

                        def win(t):
                            return t[:, 0:nz, XLo:XHo]

                        nby = yv[:, 0:nz, XLo + dx:XHo + dx]
                        nbz = ZV[dz][:, 0:nz, XLo + dx:XHo + dx]
                        dX = tmp2("dX"); dY_ = tmp2("dY"); dZ = tmp2("dZ")
                        nc.vector.tensor_sub(
                            win(dX), C["pxm16"][:, zc:zc + nz, XLo:XHo],
                            PV[dx][:, 0:nz, XLo + dx:XHo + dx])
                        nc.vector.tensor_sub(
                            win(dY_), C["yg16"][:, zc:zc + nz, XLo:XHo], nby)
                        nc.vector.tensor_sub(
                            win(dZ), C["zg16"][:, zc:zc + nz, XLo:XHo], nbz)

                        # r2 = dX^2+dY^2+dZ^2 (squares on ACT, adds on DVE)
                        x2 = tmpB("x2"); y2 = tmpB("y2"); z2 = tmp("z2")
                        nc.scalar.activation(win(x2), win(dX), Act.Square)
                        nc.scalar.activation(win(y2), win(dY_), Act.Square)
                        nc.scalar.activation(win(z2), win(dZ), Act.Square)
                        r2a = tmp("r2a")
                        nc.vector.tensor_add(win(r2a), win(x2), win(y2))
                        r2 = tmp("r2")
                        nc.vector.tensor_add(win(r2), win(r2a), win(z2))
                        # invK = KNCS/r in one ACT op: Rsqrt(r2/SCL + eps)
                        # (eps keeps r2==0 finite; G*d is then big*0 = 0)
                        invK = tmpB("invK")
                        nc.scalar.activation(win(invK), win(r2),
                                             Act.Abs_reciprocal_sqrt,
                                             scale=float(1.0 / SCL), bias=1e-30)

                        # velocity diffs on Pool, products + partial sums on DVE
                        dvx = tmpP("dvx"); dvy = tmpP("dvy"); dvz = tmpP("dvz")
                        nc.gpsimd.tensor_tensor(win(dvx), cw(C["vxe"]),
                                                nwv("vxe"), op=Alu.subtract)
                        nc.gpsimd.tensor_tensor(win(dvy), cw(C["vye"]),
                                                nwv("vye"), op=Alu.subtract)
                        nc.gpsimd.tensor_tensor(win(dvz), cw(C["vze"]),
                                                nwv("vze"), op=Alu.subtract)
                        vp1 = tmp2("vp1")
                        nc.vector.tensor_mul(win(vp1), win(dvx), win(dX))
                        dv = tmp("dv")
                        nc.vector.tensor_mul(win(dv), win(dvy), win(dY_))
                        nc.vector.tensor_add(win(vp1), win(vp1), win(dv))
                        dv2 = tmp("dv2")
                        nc.vector.tensor_mul(win(dv2), win(dvz), win(dZ))
                        nc.vector.tensor_add(win(vp1), win(vp1), win(dv2))
                        # G = relu(invK-KN) - (vd*invK^2/SCL)*(invK>KN),
                        # decomposed: spring relu + inv2 on ACT, gate via
                        # 4x tensor_scalar, combine on DVE
                        S = tmpB("x2")
                        nc.scalar.activation(win(S), win(invK), Act.Relu,
                                             bias=float(-KN))
                        inv2 = tmp("z2")
                        nc.scalar.activation(win(inv2), win(invK), Act.Square,
                                             scale=float(1.0 / math.sqrt(SCL)))
                        g8 = tmp("dv")
                        nc.vector.tensor_scalar(win(g8), win(invK), KN, None,
                                                op0=Alu.is_gt)
                        i2g = tmp("dv2")
                        nc.vector.tensor_mul(win(i2g), win(inv2), win(g8))
                        D = tmpB("y2")
                        nc.vector.tensor_mul(win(D), win(vp1), win(i2g))
                        G = tmp("G")
                        nc.vector.tensor_sub(win(G), win(S), win(D))

                        tX = tmpB("tX"); tY = tmpB("tY"); tZ = tmpB("tZ")
                        nc.vector.tensor_mul(win(tX), win(G), win(dX))
                        nc.vector.tensor_mul(win(tY), win(G), win(dY_))
                        nc.gpsimd.tensor_tensor(win(tZ), win(G), win(dZ),
                                                op=Alu.mult)

                        last = n_done == N_OFFS - 1
                        for (F, t) in ((FX, tX), (FY, tY), (FZ, tZ)):
                            for half in (0, 1):
                                zr = 4 * half
                                key = (id(F), half)
                                st = key not in started
                                started.add(key)
                                # direct: F[y,z',x'] += t[y, z'+dz, x'+4]
                                nc.tensor.matmul(
                                    F[:, zr:zr + 4, :],
                                    idt[:, 0, :],
                                    t[:, dz + zr:dz + zr + 4, 4:132],
                                    start=st, stop=False)
                                # scatter: F[y,z',x'] -= t[y-dy, z', x'+4-dx]
                                nc.tensor.matmul(
                                    F[:, zr:zr + 4, :],
                                    idt[:, 1 + (dy + 2), :],
                                    t[:, zr:zr + 4, 4 - dx:132 - dx],
                                    start=False, stop=last)
                        n_done += 1

                # ---- integration (pos/vel f32 loaded on demand) ----
                zv_ = slice(0, 8)
                xs = slice(4, 4 + X)
                mw = C["mg"][:, zv_, xs]
                for ci, (F, vname, pname) in enumerate(
                        ((FX, "vx", "pxm"), (FY, "vy", "yg"), (FZ, "vz", "zg"))):
                    vt = stpool.tile([128, BS, X], F32, tag="vt")
                    nc.sync.dma_start(
                        out=vt[:],
                        in_=d[vname][2:130, 8 * b + 2:8 * b + 10, 4:132])
                    pt = stpool.tile([128, BS, X], F32, tag="pt")
                    nc.sync.dma_start(
                        out=pt[:],
                        in_=d[pname][2:130, 8 * b + 2:8 * b + 10, 4:132])
                    scr = stpool.tile([128, BS, X], F32, tag="scr")
                    nc.vector.scalar_tensor_tensor(
                        scr[:], F[:], C1, vt[:],
                        op0=Alu.mult, op1=Alu.add)
                    stv = stpool.tile([128, BS, X], F32, tag="stg")
                    if ci == 2:
                        nc.vector.scalar_tensor_tensor(
                            stv[:], scr[:], GRAV * DT, mw,
                            op0=Alu.subtract, op1=Alu.mult)
                    else:
                        nc.vector.tensor_mul(stv[:], scr[:], mw)
                    nc.sync.dma_start(out=out_d[3 + ci, :, 8 * b:8 * b + BS, :],
                                      in_=stv[:])
                    scr2 = stpool.tile([128, BS, X], F32, tag="scr")
                    nc.vector.scalar_tensor_tensor(
                        scr2[:], stv[:], DT, pt[:],
                        op0=Alu.mult, op1=Alu.add)
                    stp = stpool.tile([128, BS, X], F32, tag="stg")
                    nc.vector.tensor_mul(stp[:], scr2[:], mw)
                    nc.sync.dma_start(out=out_d[ci, :, 8 * b:8 * b + BS, :],
                                      in_=stp[:])
    nc.compile()
    return nc


_NC_CACHE = []


def get_nc():
    if not _NC_CACHE:
        _NC_CACHE.append(build_nc())
    return _NC_CACHE[0]


def assemble(core_outs):
    full = np.concatenate([o.transpose(0, 2, 1, 3) for o in core_outs], axis=1)
    return full.reshape(6, 1, 1, NZ_G, Y, X)


def kernel(**inputs):
    nc = get_nc()
    in_maps = host_prepare(inputs)
    res = run_bass_kernel_spmd(nc, in_maps, core_ids=list(range(N_CORES)))
    return assemble([res.results[c]["out"] for c in range(N_CORES)])


if __name__ == "__main__":
    rng = np.random.default_rng(0)
    shp = (1, 1, NZ_G, Y, X)
    inputs = {
        "x_grid": rng.random(shp, dtype=np.float32),
        "y_grid": rng.random(shp, dtype=np.float32),
        "z_grid": rng.random(shp, dtype=np.float32),
        "vx_grid": rng.standard_normal(shp, dtype=np.float32),
        "vy_grid": rng.standard_normal(shp, dtype=np.float32),
        "vz_grid": rng.standard_normal(shp, dtype=np.float32),
        "mask_grid": np.ones(shp, dtype=np.float32),
    }
    out = kernel(**inputs)
    print("out shape:", out.shape, "finite:", np.isfinite(out).all())

